# revision 33
# baseline (speedup 1.0000x reference)
"""Trainium2 Bass kernel for MultiHeadDilatedAttention.

Full inputs in, full output out. Sharding: 8 cores = (batch b in 0..3) x
(segment-position half). Each (b, s) pair is an independent attention problem
(attention runs across segments n at fixed position-in-segment s), so each
core handles b = c//2 and 64 of the 128 s values. No collectives: the output
rows t = s*64 + o for a core's s-range form a contiguous chunk of y[b].

Design (v16, 169 us vs 321 us baseline):
  - x cast to bf16 on host, HWDGE-loaded in 4 row-blocks (small first
    block) so QKV chains start ~11 us in; weight DMA split per (h,p)
    piece in consumption order on the second HWDGE queue.
  - phase B: QKV projection chains at full PE clock (216 ns / 512-col
    matmul, LDWEIGHTS hidden); PSUM evictions round-robin vector/scalar.
  - V^T -> V-natural via PE transposes emitted before the final Q/K
    chains so their PSUM->SBUF copies drain during phase B. For L<32
    heads, slots stay 32-aligned via window-shifted transposes; slot
    garbage rows are neutralized by exact-zero smKQ rows (PSUM memset
    to -3e10 + 1e-30 softmax-denominator epsilon).
  - phase D in 16-s chunks: KQ+softmax for all heads, att matmuls lag
    two chunks behind so the PE never waits on the DVE softmax chain.
    softmax reduce/mul on vector, exp on scalar, scatters alternating.
    att results scattered into compact per-head atT (col = s*L + l).
  - phase E (all 4 chunks back to back): out-projection exploits concat
    sparsity - output offset o only receives heads with dil_h | o, rows
    grouped into 4 offset-classes, each class chain contracts only its
    contributing heads (2.13x fewer PE cycles than dense). Per-half
    1-bank PSUM tiles (8-deep rotation), evict halves on vector+scalar
    in parallel, contiguous 256 KB bf16 stores alternating both HWDGE
    queues; host inverse-permutes rows and upcasts to f32.

Known pitfalls encoded here: GPSIMD cannot touch PSUM; DMA-XBAR
transpose is both slow and signals its semaphore before data lands
(race); the PE p-state ramp (0.65/1.2/2.4 GHz) makes every cross-engine
stall cost ~2x for the next 3 us, so phases are organized to give the
PE long uninterrupted streaks.
"""

import numpy as np
import ml_dtypes
from contextlib import ExitStack

import concourse.bass as bass
import concourse.mybir as mybir
import concourse.tile as tile
from concourse import bacc
from concourse.masks import make_identity
from concourse.bass_utils import run_bass_kernel_spmd

F32 = mybir.dt.float32
BF16 = mybir.dt.bfloat16
AX = mybir.AxisListType

B, T, E = 4, 8192, 1024
SEG = 128          # segment size (= #s positions overall)
NB = T // SEG      # 64 segments (attention length before dilation)
NS = 64            # s values per core
ROWS = NB * NS     # 4096 rows per core
NXQ = 4            # x row-blocks (non-uniform: small first block)
XBN = [8, 16, 16, 24]          # n (l-block) counts per x block
XB0 = [0, 8, 24, 40]           # starting n of each block
QROWS = ROWS // NXQ  # legacy name (unused for shapes)
DK = 128
H = 4
DILS = [1, 2, 4, 8]
LS = [NB // d for d in DILS]       # [64, 32, 16, 8]
CONTR = [64, 32, 32, 32]           # att contraction rows (slot height)
G = [2, 4, 4, 4]                   # partition slots used per head
SLOT = [64, 32, 32, 32]            # slot stride
MG = [8, 4, 4, 4]                  # m-groups per KQ psum tile (16 s/chunk)
MOFF = [0, 64, 96, 112]            # mask column offsets, widths LS
VPAD = [0, 0, 16, 24]              # vt column padding for shifted windows
NG = [32, 16, 16, 16]              # vnat [128,128] groups per head
NORM = float(1.0 / np.sqrt(DK))
NEG = -1.0e10
PSNEG = -3.0e10
NECHUNK = E // 128                 # 8
USE_DMA_TRANSPOSE = False          # XBAR transpose is slow + racy
SCHUNK = 16                        # s values per phase-D/E chunk
NCHUNK = NS // SCHUNK              # 4

# out-projection classes: (o0, ostep, o-count, heads, s-per-tile)
CLASSES = [
    (1, 2, 32, (0,), 4),           # o odd           -> head 0 only
    (2, 4, 16, (0, 1), 8),         # o = 2 mod 4     -> heads 0,1
    (4, 8, 8, (0, 1, 2), 16),      # o = 4 mod 8     -> heads 0,1,2
    (0, 8, 8, (0, 1, 2, 3), 16),   # o = 0 mod 8     -> all heads
]


def _vnat_loc(h, s):
    """(group, slot) of V_s inside vnat[h]."""
    if h == 0:
        return s // 2, (s % 2) * 64
    if h == 1:
        return s // 4, (s % 4) * 32
    if h == 2:
        return (s // 8) * 2 + (s % 2), ((s % 8) // 2) * 32
    return (s // 16) * 4 + (s % 4), ((s % 16) // 4) * 32


def _slist(h, pi, S0):
    """s values (ci order) handled by partition-slot pi in chunk S0."""
    if h == 0:
        return [S0 + ci * 2 + pi for ci in range(8)]
    if h == 1:
        return [S0 + ci * 4 + pi for ci in range(4)]
    if h == 2:
        return [S0 + 2 * pi + j8 * 8 + j1 for j8 in (0, 1) for j1 in (0, 1)]
    return [S0 + 4 * pi + j for j in range(4)]


def build_program(bias_zero: bool = True, parts=("c", "d", "e")) -> bass.Bass:
    nc = bacc.Bacc("TRN2", target_bir_lowering=False, debug=False)
    xs = nc.dram_tensor("xs", [NECHUNK * 128 * ROWS], BF16,
                        kind="ExternalInput").ap()
    wqkv = nc.dram_tensor("wqkv", [128, 12 * NECHUNK * 128], BF16,
                          kind="ExternalInput").ap()
    wout = nc.dram_tensor("wout", [128, H * E], BF16, kind="ExternalInput").ap()
    maskd = nc.dram_tensor("masks", [128, 120], F32, kind="ExternalInput").ap()
    biasd = nc.dram_tensor("bias", [128, E], F32, kind="ExternalInput").ap()
    y = nc.dram_tensor("y", [ROWS, E], BF16, kind="ExternalOutput").ap()
    dbg = None
    if "dbg" in parts:
        dbg = {
            "at": nc.dram_tensor("dbg_at", [H, 128, NS * 64], BF16,
                                 kind="ExternalOutput").ap(),
            "vn": nc.dram_tensor("dbg_vn", [H, 128, 32 * 128], BF16,
                                 kind="ExternalOutput").ap(),
            "qk": nc.dram_tensor("dbg_qk", [H, 2, 128, 64 * 64 + 24], BF16,
                                 kind="ExternalOutput").ap(),
        }

    _build_body(nc, xs, wqkv, wout, maskd, biasd, y, bias_zero, parts, dbg)
    nc.finalize()
    return nc


def _build_body(nc, xs, wqkv, wout, maskd, biasd, y, bias_zero, parts=("c", "d", "e"), dbg=None):
    with ExitStack() as ctx:
        tc = ctx.enter_context(tile.TileContext(nc))

        persist = ctx.enter_context(tc.tile_pool(name="persist", bufs=1))
        ident = None
        if not USE_DMA_TRANSPOSE:
            ident = persist.tile([128, 128], BF16, tag="ident")
            make_identity(nc, ident)
        wout_sb = persist.tile([128, H * E], BF16, tag="wout_sb")
        mask_sb = persist.tile([128, 120], F32, tag="mask_sb")
        bias_sb = None
        if not bias_zero:
            bias_sb = persist.tile([128, E], F32, tag="bias_sb")

        # persistent per-head tensors
        qkvpool = ctx.enter_context(tc.tile_pool(name="qkv", bufs=1))
        qkv_sb = {}
        for h in range(H):
            for p in range(3):
                pad = VPAD[h] if p == 2 else 0
                qkv_sb[(h, p)] = qkvpool.tile(
                    [128, LS[h] * NS + pad], BF16,
                    tag=f"qkv{h}{p}", name=f"qkv{h}{p}")
        vnatpool = ctx.enter_context(tc.tile_pool(name="vnat", bufs=1))
        vnat = [vnatpool.tile([128, NG[h] * 128], BF16, tag=f"vnat{h}",
                              name=f"vnat{h}") for h in range(H)]
        atpool = ctx.enter_context(tc.tile_pool(name="atT", bufs=1))
        atT = [atpool.tile([128, LS[h] * NS], BF16, tag=f"atT{h}",
                           name=f"atT{h}") for h in range(H)]

        # ---- phase A: queue DMAs (x on sync queue, weights on scalar) ----
        w_pool = ctx.enter_context(tc.tile_pool(name="w", bufs=1))
        w_sb = w_pool.tile([128, 12 * NECHUNK * 128], BF16, tag="w_sb")
        with ExitStack() as pctx:
            xt_pool = pctx.enter_context(tc.tile_pool(name="xt", bufs=1))
            xt = [xt_pool.tile([128, NECHUNK * XBN[q] * NS], BF16,
                               tag=f"xt{q}", name=f"xt{q}")
                  for q in range(NXQ)]
            # xs host layout: [block, ec, 128, block_rows] flattened
            xoff = 0
            for q in range(NXQ):
                brows = XBN[q] * NS
                for ec in range(NECHUNK):
                    nc.sync.dma_start(
                        out=xt[q][:, ec * brows:(ec + 1) * brows],
                        in_=xs[xoff:xoff + 128 * brows].rearrange(
                            "(p c) -> p c", p=128))
                    xoff += 128 * brows
            WCH = NECHUNK * 128
            for h in range(H):
                for p in (2, 0, 1):
                    wi = (h * 3 + p) * WCH
                    nc.scalar.dma_start(out=w_sb[:, wi:wi + WCH],
                                        in_=wqkv[:, wi:wi + WCH])
            nc.scalar.dma_start(out=mask_sb, in_=maskd)
            nc.scalar.dma_start(out=wout_sb, in_=wout)
            if not bias_zero:
                nc.scalar.dma_start(out=bias_sb, in_=biasd)
            # init vt pad columns (read by shifted transpose windows)
            for h in (2, 3):
                L = LS[h]
                nc.gpsimd.memset(qkv_sb[(h, 2)][:, L * NS:], 0.0)

            # ---- phase B: QKV projection, half by half -------------------
            qk_ps = pctx.enter_context(
                tc.tile_pool(name="qk_ps", bufs=6, space="PSUM"))
            vt_ps = pctx.enter_context(
                tc.tile_pool(name="vt_ps", bufs=2, space="PSUM"))
            ev_engines = [nc.vector.tensor_copy, nc.scalar.copy]
            bstate = {"ev": 0, "tq": 0}

            def qkv_chains(q, h, p):
                L, dil = LS[h], DILS[h]
                lcnt = XBN[q] // dil
                lbase = XB0[q] // dil
                ncols = lcnt * NS
                xt_r = xt[q].rearrange("p (e n s) -> p e n s",
                                       e=NECHUNK, s=NS)
                dst = qkv_sb[(h, p)]
                for nt in range((ncols + 511) // 512):
                    cw = min(512, ncols - nt * 512)
                    nl = cw // NS
                    l0 = nt * (512 // NS)
                    ps = qk_ps.tile([128, 512], F32)
                    for ec in range(NECHUNK):
                        wi = ((h * 3 + p) * NECHUNK + ec) * 128
                        rhs = xt_r[:, ec, l0 * dil:(l0 + nl) * dil:dil, :]
                        nc.tensor.matmul(
                            ps[:, :cw], w_sb[:, wi:wi + 128], rhs,
                            start=(ec == 0), stop=(ec == NECHUNK - 1))
                    gl0 = lbase + l0
                    ev = ev_engines[bstate["ev"] % 2]
                    bstate["ev"] += 1
                    if p == 2:
                        # V^T stored s-major (col = s*L + l)
                        out_ap = dst[:, :L * NS].rearrange(
                            "p (s l) -> p l s", l=L)[:, gl0:gl0 + nl, :]
                        in_ap = ps[:, :cw].rearrange(
                            "p (l s) -> p l s", s=NS)
                        ev(out=out_ap, in_=in_ap)
                    else:
                        c0 = gl0 * NS
                        ev(out=dst[:, c0:c0 + cw], in_=ps[:, :cw])

            def vtranspose(S0):
                # V^T windows -> V natural for chunk S0's s values.
                for h in range(H):
                    L = LS[h]
                    vt = qkv_sb[(h, 2)]
                    if h == 0:
                        g0 = S0 // 2
                        c0s = [g * 2 * L for g in range(g0, g0 + 8)]
                    elif h == 1:
                        g0 = S0 // 4
                        c0s = [g * 4 * L for g in range(g0, g0 + 4)]
                    elif h == 2:
                        g0 = (S0 // 8) * 2
                        c0s = [(S0 + w * 8 + k) * L
                               for w in range(2) for k in range(2)]
                    else:
                        g0 = (S0 // 16) * 4
                        c0s = [(S0 + k) * L for k in range(4)]
                    if USE_DMA_TRANSPOSE:
                        for i, c0 in enumerate(c0s):
                            eng = [nc.sync, nc.scalar][bstate["tq"] % 2]
                            bstate["tq"] += 1
                            eng.dma_start(
                                out=vnat[h][:, (g0 + i) * 128:
                                            (g0 + i + 1) * 128],
                                in_=vt[:, c0:c0 + 128], transpose=True)
                        continue
                    ng = len(c0s)
                    pt = vt_ps.tile([128, 1024], BF16, tag="vt")
                    for i, c0 in enumerate(c0s):
                        nc.tensor.transpose(pt[:, i * 128:(i + 1) * 128],
                                            vt[:, c0:c0 + 128], ident)
                    cp = ev_engines[bstate["tq"] % 2]
                    bstate["tq"] += 1
                    cp(out=vnat[h][:, g0 * 128:(g0 + ng) * 128],
                       in_=pt[:, :ng * 128])

            for q in range(NXQ):
                for h in range(H):
                    for p in ((2, 0, 1) if q < NXQ - 1 else (2,)):
                        qkv_chains(q, h, p)
            # all V^T complete: transpose now so the copies drain during
            # the remaining Q/K chains and phase D never waits on vnat
            for ck in range(NCHUNK):
                vtranspose(ck * SCHUNK)
            for h in range(H):
                for p in (0, 1):
                    qkv_chains(NXQ - 1, h, p)

            if "d" not in parts:
                # dummy y write so partial variants have a defined output
                nc.sync.dma_start(out=y[0:128, :],
                                  in_=w_sb[:, 0:E])
        if "c" not in parts or "d" not in parts:
            return
        # ---- phase D+E: attention + out-projection, 16-s chunks ----------
        with ExitStack() as pctx:
            dctx = ExitStack()
            kq_ps = dctx.enter_context(
                tc.tile_pool(name="kq_ps", bufs=4, space="PSUM"))
            at_ps = dctx.enter_context(
                tc.tile_pool(name="at_ps", bufs=4, space="PSUM"))
            sm_pool = pctx.enter_context(tc.tile_pool(name="sm", bufs=3))
            small = pctx.enter_context(tc.tile_pool(name="small", bufs=4))
            yo_pool = pctx.enter_context(tc.tile_pool(name="y_sb", bufs=6))
            sc_engines = [nc.vector.tensor_copy, nc.scalar.copy]
            state = {"sc": 0, "ev": 0, "tq": 0, "st": 0, "rowblk": 0}

            def kq_softmax(S0):
                # KQ + softmax for all heads; returns live smkq tiles.
                smkqs = {}
                for h in range(H):
                    L, g, sl, mg, cl = LS[h], G[h], SLOT[h], MG[h], CONTR[h]
                    kt_r = qkv_sb[(h, 1)].rearrange("p (l s) -> p l s", s=NS)
                    qt_r = qkv_sb[(h, 0)].rearrange("p (l s) -> p l s", s=NS)
                    m_sl = mask_sb[:, MOFF[h]:MOFF[h] + L]
                    ps_kq = kq_ps.tile([128, mg * L], F32, tag="kq")
                    if cl > L:
                        # neutralize slot garbage rows -> exp == 0 (matmuls
                        # overwrite the real rows below)
                        nc.vector.memset(ps_kq, PSNEG)
                    for ci in range(mg):
                        for pi in range(g):
                            s = _slist(h, pi, S0)[ci]
                            nc.tensor.matmul(
                                ps_kq[pi * sl:pi * sl + L,
                                      ci * L:(ci + 1) * L],
                                kt_r[:, :, s], qt_r[:, :, s],
                                start=True, stop=True,
                                tile_position=(0, pi * sl))
                    numer = sm_pool.tile([128, mg * L], F32, tag="numer")
                    enumer = sm_pool.tile([128, mg * L], BF16, tag="enumer")
                    sums = small.tile([128, mg], F32, tag="sums")
                    recip = small.tile([128, mg], F32, tag="recip")
                    smkq = sm_pool.tile([128, mg * L], BF16, tag="smkq",
                                        bufs=12)
                    mask_bc = bass.AP(tensor=m_sl.tensor, offset=m_sl.offset,
                                      ap=[m_sl.ap[0], [0, mg], m_sl.ap[1]])
                    nc.vector.tensor_add(numer, ps_kq, mask_bc)
                    nc.scalar.activation(
                        enumer, numer,
                        mybir.ActivationFunctionType.Exp, scale=NORM)
                    nc.vector.reduce_sum(
                        sums, enumer.rearrange("p (c l) -> p c l", l=L),
                        axis=AX.X)
                    if cl > L:
                        nc.gpsimd.tensor_scalar_add(sums, sums, 1e-30)
                    nc.vector.reciprocal(recip, sums)
                    rc_bc = bass.AP(tensor=recip.tensor, offset=recip.offset,
                                    ap=[recip.ap[0], [1, mg], [0, L]])
                    nc.vector.tensor_mul(smkq, enumer, rc_bc)
                    smkqs[h] = smkq
                return smkqs

            def att_tile(S0, smkqs, h, pi):
                L, g, sl, mg, cl = LS[h], G[h], SLOT[h], MG[h], CONTR[h]
                smkq = smkqs[h]
                dil = DILS[h]
                slot = pi * sl
                ps_at = at_ps.tile([128, 512], F32, tag="at")
                for ci in range(mg):
                    s = _slist(h, pi, S0)[ci]
                    gi, vslot = _vnat_loc(h, s)
                    assert vslot == slot
                    lhsT = vnat[h][slot:slot + cl,
                                   gi * 128:(gi + 1) * 128]
                    rhs = smkq[slot:slot + cl, ci * L:(ci + 1) * L]
                    nc.tensor.matmul(
                        ps_at[:, ci * L:(ci + 1) * L], lhsT, rhs,
                        start=True, stop=True,
                        tile_position=(slot, 0))
                # scatter into compact atT[h] at cols s*L + l
                L_ = L
                at_r = atT[h].rearrange("p (s l) -> p s l", l=L_)
                ps_r = ps_at[:, :mg * L]
                if h == 0:
                    in_ap = ps_r.rearrange("p (c l) -> p c l", l=L)
                    out_ap = at_r[:, S0 + pi:S0 + pi + 15:2, :]
                elif h == 1:
                    in_ap = ps_r.rearrange("p (c l) -> p c l", l=L)
                    out_ap = at_r[:, S0 + pi:S0 + pi + 13:4, :]
                elif h == 2:
                    # ci = j8*2 + j1 -> s = S0 + j8*8 + 2*pi + j1
                    in_ap = ps_r.rearrange(
                        "p (j8 j1 l) -> p j8 j1 l", j8=2, j1=2)
                    out_ap = atT[h].rearrange(
                        "p (s2 s1 l) -> p s2 s1 l", s1=8, l=L)[
                        :, S0 // 8:S0 // 8 + 2, 2 * pi:2 * pi + 2, :]
                else:
                    in_ap = ps_r.rearrange("p (c l) -> p c l", l=L)
                    out_ap = at_r[:, S0 + 4 * pi:S0 + 4 * pi + 4, :]
                eng = sc_engines[state["sc"] % 2]
                state["sc"] += 1
                eng(out=out_ap, in_=in_ap)

            def outproj_tile(S0, o0, ostep, no, heads, ns, k, rowblk):
                s0 = S0 + k * ns
                halves = []
                for half in range(2):
                    cs = half * 512
                    ps_y = state["y_ps"].tile([128, 512], F32, tag="y")
                    halves.append(ps_y)
                    for idx, h in enumerate(heads):
                        L, dil = LS[h], DILS[h]
                        l0, lstep = o0 // dil, max(ostep // dil, 1)
                        lhsT = atT[h].rearrange(
                            "p (s l) -> p s l", l=L)[
                            :, s0:s0 + ns,
                            l0:l0 + (no - 1) * lstep + 1:lstep]
                        nc.tensor.matmul(
                            ps_y, lhsT,
                            wout_sb[:, h * E + cs:h * E + cs + 512],
                            start=(idx == 0),
                            stop=(idx == len(heads) - 1))
                y_sb = yo_pool.tile([128, E], BF16)
                if bias_zero:
                    nc.vector.tensor_copy(out=y_sb[:, :512],
                                          in_=halves[0])
                    nc.scalar.copy(out=y_sb[:, 512:], in_=halves[1])
                else:
                    nc.vector.tensor_add(y_sb[:, :512], halves[0],
                                         bias_sb[:, :512])
                    nc.vector.tensor_add(y_sb[:, 512:], halves[1],
                                         bias_sb[:, 512:])
                # contiguous 256 KB store; host un-permutes rows
                eng = nc.scalar if state["st"] % 2 else nc.sync
                state["st"] += 1
                eng.dma_start(out=y[rowblk * 128:(rowblk + 1) * 128, :],
                              in_=y_sb)

            def outproj_args(S0):
                out = []
                for o0, ostep, no, heads, ns in CLASSES:
                    for k in range(SCHUNK // ns):
                        rowblk = state["rowblk"]
                        state["rowblk"] += 1
                        out.append((S0, o0, ostep, no, heads, ns, k, rowblk))
                return out

            for ck in range(NCHUNK):
                S0 = ck * SCHUNK
                smk = kq_softmax(S0)
                for h in range(H):
                    for pi in range(G[h]):
                        att_tile(S0, smk, h, pi)
            dctx.close()
            if "e" in parts:
                with ExitStack() as ectx:
                    y_ps = ectx.enter_context(
                        tc.tile_pool(name="y_ps", bufs=8, space="PSUM"))
                    state["y_ps"] = y_ps
                    for ck in range(NCHUNK):
                        for args in outproj_args(ck * SCHUNK):
                            outproj_tile(*args)
            if dbg is not None:
                for h in range(H):
                    nc.sync.dma_start(out=dbg["at"][h], in_=atT[h])
                    nc.sync.dma_start(out=dbg["vn"][h][:, :NG[h] * 128],
                                      in_=vnat[h])
                    for p in range(2):
                        w = LS[h] * NS + (VPAD[h] if p == 1 else 0)
                        nc.sync.dma_start(
                            out=dbg["qk"][h, p][:, :w],
                            in_=qkv_sb[(h, 2 * p)][:, :w])
    nc.finalize()
    return nc


_NC = {}


def _get_program(bias_zero=True):
    if bias_zero not in _NC:
        _NC[bias_zero] = build_program(bias_zero)
    return _NC[bias_zero]


def _host_inputs(Wk, Wq, Wv, W_out, b_out):
    bf = ml_dtypes.bfloat16
    Wstack = np.stack([Wq, Wk, Wv], 1)                     # [H, 3, 128, 1024]
    tmp = Wstack.reshape(H, 3, 128, NECHUNK, 128)          # [h, p, c, ec, r]
    wqkv_sb = np.ascontiguousarray(
        tmp.transpose(4, 0, 1, 3, 2)).reshape(128, -1).astype(bf)
    wout_sb = np.ascontiguousarray(
        W_out.reshape(E, H, 128).transpose(2, 1, 0)).reshape(128, H * E
                                                             ).astype(bf)
    mask_host = np.full((128, 120), NEG, np.float32)
    for h in range(H):
        L, sl = LS[h], SLOT[h]
        for p in range(128):
            n = p % sl
            if n < L:
                mask_host[p, MOFF[h]:MOFF[h] + n + 1] = 0.0
    bias_sb = np.ascontiguousarray(
        np.broadcast_to(np.asarray(b_out, np.float32).reshape(1, E),
                        (128, E)))
    return wqkv_sb, wout_sb, mask_host, bias_sb


def _shard_x(x16, c):
    """x16: bf16 [B, T, E]. Flat device layout: per block [ec, 128, rows]."""
    b, half = c // 2, c % 2
    xs = x16[b].reshape(NB, SEG, E)[:, half * NS:(half + 1) * NS, :]
    xs = xs.reshape(ROWS, NECHUNK, 128)            # [row, ec, e]
    parts = []
    for q in range(NXQ):
        blk = xs[XB0[q] * NS:(XB0[q] + XBN[q]) * NS]   # [brows, ec, 128]
        parts.append(np.ascontiguousarray(blk.transpose(1, 2, 0)).reshape(-1))
    return np.concatenate(parts)


def _row_perm():
    """Device y row order -> local t. Inverse-applied in assemble()."""
    perm = []
    for ck in range(NCHUNK):
        S0 = ck * SCHUNK
        for o0, ostep, no, heads, ns in CLASSES:
            for k in range(SCHUNK // ns):
                for s in range(S0 + k * ns, S0 + (k + 1) * ns):
                    for oi in range(no):
                        perm.append(s * 64 + o0 + oi * ostep)
    return np.asarray(perm)


_PERM = _row_perm()


def prepare(x, Wk, Wq, Wv, W_out, b_out):
    x16 = np.asarray(x, np.float32).astype(ml_dtypes.bfloat16)
    wqkv_sb, wout_sb, mask_host, bias_sb = _host_inputs(
        np.asarray(Wk, np.float32), np.asarray(Wq, np.float32),
        np.asarray(Wv, np.float32), np.asarray(W_out, np.float32),
        np.asarray(b_out, np.float32))
    bias_zero = not np.any(np.asarray(b_out))
    in_maps = []
    for c in range(8):
        in_maps.append({"xs": _shard_x(x16, c), "wqkv": wqkv_sb,
                        "wout": wout_sb, "masks": mask_host,
                        "bias": bias_sb})
    return in_maps, bias_zero


def assemble(res):
    y = np.empty((B, T, E), np.float32)
    for c in range(8):
        b, half = c // 2, c % 2
        yc = np.asarray(res.results[c]["y"], dtype=np.float32)
        dst = y[b, half * ROWS:(half + 1) * ROWS, :]
        dst[_PERM] = yc
    return y


def kernel(x, Wk, Wq, Wv, W_out, b_out):
    in_maps, bias_zero = prepare(x, Wk, Wq, Wv, W_out, b_out)
    nc = _get_program(bias_zero)
    res = run_bass_kernel_spmd(nc, in_maps, core_ids=list(range(8)))
    return assemble(res)
